# revision 1
# baseline (speedup 1.0000x reference)
"""Causal multi-head attention (QKV-packed) on 8 Trainium2 NeuronCores.

Sharding: pure head-parallel. B*H = 32 (batch, head) pairs -> 4 per core,
zero inter-core communication. Per head, flash-style causal attention is
computed entirely in the "transposed" orientation so no on-device
transposes are needed:

  - Host pre-lays-out Q^T, K^T as [D=128, S] (D on partitions) and V as
    k-blocks [128, D]; scores are computed transposed S_T[k, q] =
    (K^T_j).T @ Q^T, softmax numerator P_T = exp(scale * S_T + mask) on
    the ACT engine, then O^T[d, q] += V_j.T @ P_T accumulates in PSUM.
    The softmax denominator comes from a ones-vector matmul over P_T,
    also PSUM-accumulated; normalization is a K=1 broadcast matmul of
    the reciprocal plus one DVE multiply. Output is returned as O^T and
    un-transposed on the host.
  - All matmuls run as float32r (full-rate fp32 at free-dim >= 256).
  - exp() skips max-subtraction: scores are ~N(0,1) after 1/sqrt(D)
    scaling, so exp is safely in fp32 range.
"""

import sys

if "/opt/trn_rl_repo" not in sys.path:
    sys.path.insert(0, "/opt/trn_rl_repo")

import numpy as np

B, S, H, D = 2, 2048, 16, 128
NCORES = 8
HPC = (B * H) // NCORES  # heads per core = 4
QS = 512   # q-strip width (PSUM bank)
KB = 128   # k-block (partition dim)
NEG = -1.0e30
SCALE = 1.0 / float(np.sqrt(D))
NSTRIP = S // QS  # 4

_nc_cache = {}


def _block_geometry(s, j):
    """For q-strip s and k-block j return (off, N): the strip-local column
    range [off, off+N) of q positions this block contributes to.
    t = j - 4s is the diagonal offset; t<0 full block, t>=0 diagonal."""
    t = j - 4 * s
    if t <= 0:
        return 0, QS
    if t == 1:
        return 128, 384
    # t == 2 and t == 3 both use 256 columns (t=3 widened so the fp32r
    # matmul keeps free-dim >= 256; its extra 128 columns are fully masked)
    return 256, 256


def _build_nc():
    import concourse.bass as bass  # noqa: F401
    import concourse.mybir as mybir
    from concourse import bacc
    from concourse.tile import TileContext

    f32 = mybir.dt.float32
    f32r = mybir.dt.float32r
    Exp = mybir.ActivationFunctionType.Exp

    nc = bacc.Bacc()
    # One packed input per head (single DMA => single wait semaphore for
    # the first matmul of each head; walrus allows only one sync-wait on
    # an fp32r matmul's fused weight load). Layout per head [128, 3*S]:
    # cols [0,S) = Q^T, [S,2S) = K^T, [2S,3S) = V swizzled so column
    # block j holds the V k-block [128, D] (v[p, j*KB+d] = V[j*KB+p, d]).
    qkvT = nc.declare_dram_parameter("qkvT", [HPC, 128, 3 * S], f32r, isOutput=False)
    cst = nc.declare_dram_parameter("cst", [128, 256], f32, isOutput=False)
    ones = nc.declare_dram_parameter("ones", [128, 128], f32r, isOutput=False)
    oT = nc.declare_dram_parameter("oT", [HPC, 128, S], f32, isOutput=True)

    with TileContext(nc) as tc:
        with (
            nc.allow_low_precision(
                reason="float32r is 4-byte; reciprocal into f32r is fine"
            ),
            tc.tile_pool(name="cpool", bufs=1) as cpool,
            tc.tile_pool(name="qkpool", bufs=2) as qkpool,
            tc.tile_pool(name="ptpool", bufs=6) as ptpool,
            tc.tile_pool(name="obpool", bufs=2) as obpool,
            tc.tile_pool(name="pst", bufs=3, space="PSUM") as pst,
            tc.tile_pool(name="pso", bufs=2, space="PSUM") as pso,
            tc.tile_pool(name="psd", bufs=2, space="PSUM") as psd,
            tc.tile_pool(name="psr", bufs=1, space="PSUM") as psr,
        ):
            cst_sb = cpool.tile([128, 256], f32)
            nc.sync.dma_start(out=cst_sb[:], in_=cst[:])
            ones_sb = cpool.tile([128, 128], f32r)
            nc.sync.dma_start(out=ones_sb[:], in_=ones[:])
            tri = cst_sb[:, 0:128]       # tri[dk, c] = 0 if dk <= c else NEG
            full = cst_sb[:, 128:256]    # all NEG
            ones_col = ones_sb[:, 0:1]   # [128, 1] of 1.0
            ones_row = ones_sb[0:1, :]   # [1, 128] of 1.0

            def epilogue_rest(h, s, o_ps, recip):
                rb = psr.tile([128, QS], f32, tag="rb")
                nc.tensor.matmul(
                    rb[:], lhsT=ones_row, rhs=recip[:], start=True, stop=True
                )
                rb_sb = obpool.tile([128, QS], f32, tag="rb_sb")
                nc.vector.tensor_copy(rb_sb[:], rb[:])
                o_sb = obpool.tile([128, QS], f32, tag="o_sb")
                nc.vector.tensor_mul(o_sb[:], o_ps[:], rb_sb[:])
                nc.sync.dma_start(out=oT[h][:, QS * s : QS * (s + 1)], in_=o_sb[:])

            pending = None  # (h, s, o_ps, recip) of the previous strip
            for h in range(HPC):
                qkv_sb = qkpool.tile([128, 3 * S], f32r, tag="qkv_sb")
                if h == 0:
                    # split the first head's load so the first matmuls can
                    # start after ~0.5MB instead of the full 3MB: K^T and
                    # Q^T for strip 0 first, then V for strip 0, then rests
                    for c0, c1 in (
                        (S, S + 512),          # K^T blocks 0-3
                        (0, 512),              # Q^T strip 0
                        (2 * S, 2 * S + 512),  # V blocks 0-3
                        (512, S),              # Q^T rest
                        (S + 512, 2 * S),      # K^T rest
                        (2 * S + 512, 3 * S),  # V rest
                    ):
                        nc.sync.dma_start(
                            out=qkv_sb[:, c0:c1], in_=qkvT[h][:, c0:c1]
                        )
                else:
                    nc.sync.dma_start(out=qkv_sb[:], in_=qkvT[h])
                qt_sb = qkv_sb[:, 0:S]
                kt_sb = qkv_sb[:, S : 2 * S]
                v_sb = qkv_sb[:, 2 * S : 3 * S]

                for s in range(NSTRIP):
                    o_ps = pso.tile([128, QS], f32, tag="o_ps")
                    den_ps = psd.tile([1, QS], f32, tag="den_ps")
                    nblk = 4 * s + 4
                    for j in range(nblk):
                        t = j - 4 * s
                        off, N = _block_geometry(s, j)
                        sT = pst.tile([128, QS], f32, tag="sT")
                        nc.tensor.matmul(
                            sT[:, 0:N],
                            lhsT=kt_sb[:, KB * j : KB * (j + 1)],
                            rhs=qt_sb[:, QS * s + off : QS * s + off + N],
                            start=True,
                            stop=True,
                        )
                        if t >= 0:
                            if t == 3:
                                # strip cols [256,384) fully masked,
                                # [384,512) triangular
                                nc.vector.tensor_add(
                                    sT[:, 0:128], sT[:, 0:128], full
                                )
                                nc.vector.tensor_add(
                                    sT[:, 128:256], sT[:, 128:256], tri
                                )
                            else:
                                loc = 128 * t - off  # always 0 here
                                nc.vector.tensor_add(
                                    sT[:, loc : loc + 128], sT[:, loc : loc + 128], tri
                                )
                        pt = ptpool.tile([128, QS], f32r, tag="pt")
                        nc.scalar.activation(pt[:, 0:N], sT[:, 0:N], Exp, scale=SCALE)
                        first, last = (j == 0), (j == nblk - 1)
                        nc.tensor.matmul(
                            o_ps[:, off : off + N],
                            lhsT=v_sb[:, KB * j : KB * (j + 1)],
                            rhs=pt[:, 0:N],
                            start=first,
                            stop=last,
                        )
                        nc.tensor.matmul(
                            den_ps[:, off : off + N],
                            lhsT=ones_col,
                            rhs=pt[:, 0:N],
                            start=first,
                            stop=last,
                        )
                    # reciprocal immediately (den_ps has bufs=1; freeing it
                    # fast keeps the single den bank available), but the rest
                    # of the epilogue is software-pipelined behind the next
                    # strip's body so its PE/DVE work overlaps the matmuls
                    recip = obpool.tile([1, QS], f32r, tag="recip")
                    nc.vector.reciprocal(recip[:], den_ps[:])
                    if pending is not None:
                        epilogue_rest(*pending)
                    pending = (h, s, o_ps, recip)
            epilogue_rest(*pending)
    nc.compile()
    return nc


def get_nc():
    if "nc" not in _nc_cache:
        _nc_cache["nc"] = _build_nc()
    return _nc_cache["nc"]


def _build_const():
    dk = np.arange(128)[:, None]
    c = np.arange(128)[None, :]
    cst = np.empty((128, 256), np.float32)
    cst[:, 0:128] = np.where(dk <= c, 0.0, NEG).astype(np.float32)
    cst[:, 128:256] = NEG
    return cst


def make_in_maps(qkv):
    qkv = np.asarray(qkv, dtype=np.float32)
    cst = _build_const()
    in_maps = []
    for core in range(NCORES):
        qkvT = np.empty((HPC, 128, 3 * S), np.float32)
        for i in range(HPC):
            bh = core * HPC + i
            b, h = bh // H, bh % H
            qkvT[i, :, 0:S] = qkv[b, :, 0, h, :].T
            qkvT[i, :, S : 2 * S] = qkv[b, :, 1, h, :].T
            qkvT[i, :, 2 * S : 3 * S] = (
                qkv[b, :, 2, h, :]
                .reshape(S // KB, KB, D)
                .transpose(1, 0, 2)
                .reshape(KB, S)
            )
        in_maps.append({"qkvT": qkvT, "cst": cst,
                        "ones": np.ones((128, 128), np.float32)})
    return in_maps


def assemble_out(results):
    out = np.empty((B, S, H, D), np.float32)
    for core in range(NCORES):
        oTc = results[core]["oT"]  # [HPC, 128, S]
        for i in range(HPC):
            bh = core * HPC + i
            b, h = bh // H, bh % H
            out[b, :, h, :] = oTc[i].T
    return out


def kernel(qkv):
    from concourse.bass_utils import run_bass_kernel_spmd

    in_maps = make_in_maps(qkv)
    nc = get_nc()
    res = run_bass_kernel_spmd(nc, in_maps, list(range(NCORES)))
    return assemble_out(res.results)



# revision 21
# speedup vs baseline: 1.0261x; 1.0261x over previous
"""Causal multi-head attention (QKV-packed) on 8 Trainium2 NeuronCores.

Sharding: pure head-parallel. B*H = 32 (batch, head) pairs -> 4 per core,
zero inter-core communication. Per head, flash-style causal attention is
computed entirely in the "transposed" orientation (k on partitions) so no
on-device transposes are needed:

  - Host pre-lays-out Q^T, K^T as [D=128, S] (fp16, D on partitions) and V
    as k-blocks [128, D] (fp16). Scores for a PAIR of k-blocks land in one
    [128, 1024] PSUM tile (2 banks); one ACT instruction computes
    pt = exp(scale*s - 2) over the written extent. The -2 bias keeps
    exp <= ~45 < 240 (TRN fp8e4 max) and cancels between numerator and
    denominator. Diagonal blocks are packed contiguously (t3 at [512,640),
    t2 at [384,640)) so the ACT extent is exactly the causal column count.
  - pt is fp8e4 except strip 0's diagonal pairs (bf16): short softmax rows
    (q < ~100) lack the numerator/denominator error cancellation that
    makes fp8 safe for long rows.
  - O^T[d, q] += V_j.T @ pt accumulates in PSUM per 512-col q-strip
    (fp16 x fp8/bf16 matmuls, 1 col/cycle), then is evacuated to SBUF
    bf16 right at strip end to free the PSUM bank.
  - The softmax denominator accumulates in ONE PSUM bank per head, strip s
    on partition 32s: fp8 pairs use a DoubleRow matmul (2 cols/cycle) with
    M=97 weights carrying a single 1.0 column (DoubleRow requires
    col_grp=0xf, so the output row is selected by the weight column, not
    by PSUM col tiling); solo diagonal regions and strip 0 use normal-rate
    ones-column matmuls. One batched DVE reciprocal per head then serves
    all 4 strips ([1,512] reciprocal is ~6 cyc/elem - batching quarters
    its cost).
  - Normalization: K=1 broadcast matmul of the reciprocal row into the
    den bank, one DVE cast to bf16, one bf16 DVE multiply. Output is bf16
    (host casts to fp32). Epilogues of head h are spread across head h+1's
    strips so the DVE work does not pile up at head boundaries.
  - Zero-input warmup matmuls (no DMA dependency) run first so the PE HAM
    clock gate opens (1.2 -> 2.4 GHz) before real data arrives.
"""

import sys

if "/opt/trn_rl_repo" not in sys.path:
    sys.path.insert(0, "/opt/trn_rl_repo")

import numpy as np

B, S, H, D = 2, 2048, 16, 128
NCORES = 8
HPC = (B * H) // NCORES  # heads per core = 4
QS = 512   # q-strip width (PSUM bank)
KB = 128   # k-block (partition dim)
NEG = -1.0e30
SCALE = 1.0 / float(np.sqrt(D))
EXP_BIAS = -2.0
NSTRIP = S // QS  # 4

_nc_cache = {}


def _build_nc():
    import concourse.bass as bass  # noqa: F401
    import concourse.mybir as mybir
    from concourse import bacc
    from concourse.tile import TileContext

    f32 = mybir.dt.float32
    f16 = mybir.dt.float16
    f8 = mybir.dt.float8e4
    bf16 = mybir.dt.bfloat16
    f32r = mybir.dt.float32r
    Exp = mybir.ActivationFunctionType.Exp
    DR = mybir.MatmulPerfMode.DoubleRow

    nc = bacc.Bacc()
    # One packed input per head [128, 3*S] fp16:
    # cols [0,S) = Q^T, [S,2S) = K^T, [2S,3S) = V swizzled so column
    # block j holds the V k-block [128, D] (v[p, j*KB+d] = V[j*KB+p, d]).
    qkvT = nc.declare_dram_parameter("qkvT", [HPC, 128, 3 * S], f16, isOutput=False)
    tri_d = nc.declare_dram_parameter("tri", [128, 128], f32, isOutput=False)
    ones8_d = nc.declare_dram_parameter("ones8", [128, 512], f8, isOutput=False)
    onesv_d = nc.declare_dram_parameter("onesv", [128, 128], f32r, isOutput=False)
    oT = nc.declare_dram_parameter("oT", [HPC, 128, S], bf16, isOutput=True)

    with TileContext(nc) as tc:
        with (
            nc.allow_low_precision(reason="fp16/fp8/bf16 staging is within tolerance"),
            tc.tile_pool(name="cpool", bufs=1) as cpool,
            tc.tile_pool(name="qkpool", bufs=2) as qkpool,
            tc.tile_pool(name="ptpool", bufs=6) as ptpool,
            tc.tile_pool(name="ptbpool", bufs=2) as ptbpool,
            tc.tile_pool(name="orpool", bufs=8) as orpool,
            tc.tile_pool(name="rcpool", bufs=2) as rcpool,
            tc.tile_pool(name="obpool", bufs=4) as obpool,
            tc.tile_pool(name="scp", bufs=2, space="PSUM") as scp,
            tc.tile_pool(name="pso", bufs=2, space="PSUM") as pso,
            tc.tile_pool(name="psd", bufs=2, space="PSUM") as psd,
        ):
            tri_sb = cpool.tile([128, 128], f32)
            nc.sync.dma_start(out=tri_sb[:], in_=tri_d[:])
            ones8 = cpool.tile([128, 512], f8)
            nc.sync.dma_start(out=ones8[:], in_=ones8_d[:])
            onesv = cpool.tile([128, 128], f32r)
            nc.sync.dma_start(out=onesv[:], in_=onesv_d[:])
            biasc = cpool.tile([128, 1], f32)
            nc.gpsimd.memset(biasc[:], EXP_BIAS)
            # [128, 2, 256] view: single 1.0 column at m=96 in each half
            o83 = ones8[:, 0:512].rearrange("p (a m) -> p a m", a=2)

            # HAM warmup: zero-input matmuls with no DMA dependency keep the
            # PE busy from the end of the preamble so the clock gate opens
            # (1.2 -> 2.4 GHz) before the first real matmul.
            zsrc = cpool.tile([128, 384], f16)
            nc.scalar.memzero(zsrc[:])
            wps = scp.tile([128, 1024], f32, tag="sc")
            for w in range(14):
                nc.tensor.matmul(
                    wps[:, 0:256],
                    lhsT=zsrc[:, 0:128],
                    rhs=zsrc[:, 128:384],
                    start=True,
                    stop=True,
                )

            def emit_epilogue(ep):
                h, s, den_t, recip_t, o_raw = ep
                # broadcast recip row 32s across partitions (K=1 matmul)
                # into the (now free) den bank of head h
                nc.tensor.matmul(
                    den_t[:, :],
                    lhsT=onesv[32 * s : 32 * s + 1, 0:128],
                    rhs=recip_t[32 * s : 32 * s + 1, 0:QS],
                    start=True,
                    stop=True,
                    tile_position=(32 * s, 0),
                )
                rb_sb = obpool.tile([128, QS], bf16, tag="rb_sb")
                nc.vector.tensor_copy(rb_sb[:], den_t[:, :])
                o_sb = obpool.tile([128, QS], bf16, tag="o_sb")
                nc.vector.tensor_mul(o_sb[:], o_raw[:], rb_sb[:])
                nc.sync.dma_start(out=oT[h][:, QS * s : QS * (s + 1)], in_=o_sb[:])

            pending = []
            for h in range(HPC):
                qkv_sb = qkpool.tile([128, 3 * S], f16, tag="qkv_sb")
                if h == 0:
                    # split the first head's load so the first matmuls can
                    # start early: K^T blocks 0-3 and Q^T strip 0 first.
                    for c0, c1 in (
                        (S, S + 512),          # K^T blocks 0-3
                        (0, 512),              # Q^T strip 0
                        (2 * S, 2 * S + 512),  # V blocks 0-3
                        (512, S),              # Q^T rest
                        (S + 512, 2 * S),      # K^T rest
                        (2 * S + 512, 3 * S),  # V rest
                    ):
                        nc.sync.dma_start(
                            out=qkv_sb[:, c0:c1], in_=qkvT[h][:, c0:c1]
                        )
                else:
                    nc.sync.dma_start(out=qkv_sb[:], in_=qkvT[h])
                qt = qkv_sb[:, 0:S]
                kt = qkv_sb[:, S : 2 * S]
                vv = qkv_sb[:, 2 * S : 3 * S]

                den = psd.tile([128, QS], f32, tag="den")
                o_raws = []
                den_started = [False]

                def den_flags(s, is_last_of_strip):
                    st = not den_started[0]
                    den_started[0] = True
                    sp = (s == NSTRIP - 1) and is_last_of_strip
                    return st, sp

                for s in range(NSTRIP):
                    r = 32 * s  # den row for this strip
                    o_ps = pso.tile([128, QS], f32, tag="o_ps")
                    q0 = QS * s
                    # pairs: (jA, jB, woff, wN, colB, NB, triA, triB)
                    # jA's scores at tile [0, wN); jB's at [512+colB,
                    # 512+colB+NB) packed contiguously after A's extent for
                    # diagonal pairs; triA/triB: strip col of a 128-wide
                    # triangular mask add (None = no mask).
                    pairs = []
                    for p in range(2 * s):
                        pairs.append((2 * p, 2 * p + 1, 0, QS, 0, QS, None, None))
                    t0, t1, t2, t3 = 4 * s, 4 * s + 1, 4 * s + 2, 4 * s + 3
                    # X = (t0, t3): t0 covers [0,512), t3 at tile [512,640)
                    pairs.append((t0, t3, 0, QS, 0, 128, 0, 384))
                    # Y = (t1, t2): t1 covers [128,512) at tile [0,384),
                    # t2 at tile [512,768) (a matmul output cannot cross the
                    # PSUM bank boundary at 512; [384,512) stays stale and
                    # its exp output is never read)
                    pairs.append((t1, t2, 128, 384, 0, 256, 128, 256))

                    npair = len(pairs)
                    for pi, (jA, jB, woff, wN, colB, NB, trA, trB) in enumerate(
                        pairs
                    ):
                        diag = pi >= npair - 2
                        s0diag = diag and s == 0
                        sc = scp.tile([128, 1024], f32, tag="sc")
                        nc.tensor.matmul(
                            sc[:, 0:wN],
                            lhsT=kt[:, KB * jA : KB * (jA + 1)],
                            rhs=qt[:, q0 + woff : q0 + woff + wN],
                            start=True,
                            stop=True,
                        )
                        # B block: tile cols [512+colB, 512+colB+NB); its
                        # strip window is [woff+wNB0, woff+wN) where
                        # wNB0 = wN - NB
                        bcol = 512 + colB
                        bq = q0 + woff + wN - NB
                        nc.tensor.matmul(
                            sc[:, bcol : bcol + NB],
                            lhsT=kt[:, KB * jB : KB * (jB + 1)],
                            rhs=qt[:, bq : bq + NB],
                            start=True,
                            stop=True,
                        )
                        if trA is not None:
                            c = trA - woff
                            nc.vector.tensor_add(
                                sc[:, c : c + 128], sc[:, c : c + 128], tri_sb[:]
                            )
                        if trB is not None:
                            c = bcol
                            nc.vector.tensor_add(
                                sc[:, c : c + 128], sc[:, c : c + 128], tri_sb[:]
                            )
                        ext = bcol + NB
                        if s0diag:
                            pt = ptbpool.tile([128, 1024], bf16, tag="ptb")
                        else:
                            pt = ptpool.tile([128, 1024], f8, tag="pt")
                        nc.scalar.activation(
                            pt[:, 0:ext],
                            sc[:, 0:ext],
                            Exp,
                            bias=biasc[:],
                            scale=SCALE,
                        )
                        # PV accumulation
                        first, last = (pi == 0), (pi == npair - 1)
                        nc.tensor.matmul(
                            o_ps[:, woff : woff + wN],
                            lhsT=vv[:, KB * jA : KB * (jA + 1)],
                            rhs=pt[:, 0:wN],
                            start=first,
                            stop=False,
                        )
                        nc.tensor.matmul(
                            o_ps[:, woff + wN - NB : woff + wN],
                            lhsT=vv[:, KB * jB : KB * (jB + 1)],
                            rhs=pt[:, bcol : bcol + NB],
                            start=False,
                            stop=last,
                        )
                        # denominator, accumulated on partition 32s of the
                        # per-head den bank
                        if not diag:
                            # full pair: DoubleRow over the whole strip
                            st, sp = den_flags(s, False)
                            nc.tensor.matmul(
                                den[0:97, 0:QS],
                                lhsT=o83[:, :, 96 - r : 193 - r],
                                rhs=pt[:, 0:1024].rearrange(
                                    "p (a b) -> p a b", a=2
                                ),
                                start=st,
                                stop=sp,
                                perf_mode=DR,
                            )
                        elif s0diag:
                            # strip 0 diagonal (bf16 pt): two normal-rate
                            # ones-matmuls per pair, all onto row 0
                            st, sp = den_flags(s, False)
                            nc.tensor.matmul(
                                den[0:1, woff : woff + wN],
                                lhsT=ones8[:, 96:97],
                                rhs=pt[:, 0:wN],
                                start=st,
                                stop=False,
                            )
                            st, sp = den_flags(s, last)
                            nc.tensor.matmul(
                                den[0:1, woff + wN - NB : woff + wN],
                                lhsT=ones8[:, 96:97],
                                rhs=pt[:, bcol : bcol + NB],
                                start=False,
                                stop=sp,
                            )
                        else:
                            # diagonal pair, fp8: solo region (A only) at
                            # normal rate + overlap region as DoubleRow
                            # (pair stride bcol - (wN - NB))
                            st, sp = den_flags(s, False)
                            solo = wN - NB
                            nc.tensor.matmul(
                                den[0 : r + 1, woff : woff + solo],
                                lhsT=ones8[:, 96 - r : 97],
                                rhs=pt[:, 0:solo],
                                start=st,
                                stop=False,
                            )
                            st, sp = den_flags(s, last)
                            pr = pt[:, solo : solo + 2 * (bcol - solo)].rearrange(
                                "p (a b) -> p a b", a=2
                            )
                            nc.tensor.matmul(
                                den[0:97, woff + solo : woff + wN],
                                lhsT=o83[:, :, 96 - r : 193 - r],
                                rhs=pr[:, :, 0:NB],
                                start=False,
                                stop=sp,
                                perf_mode=DR,
                            )
                    # evacuate O^T early (frees the PSUM bank; bf16 is fine
                    # for the un-normalized accumulator)
                    o_raw = orpool.tile([128, QS], bf16, tag="o_raw")
                    nc.vector.tensor_copy(o_raw[:], o_ps[:])
                    o_raws.append(o_raw)
                    # spread the previous head's epilogues across this
                    # head's strips
                    if pending:
                        emit_epilogue(pending.pop(0))

                # one batched reciprocal per head (rows 0/32/64/96 hold the
                # 4 strips' denominators; other rows are zeros/garbage,
                # unused)
                recip = rcpool.tile([128, QS], f32r, tag="recip")
                nc.vector.reciprocal(recip[:], den[:])
                for s in range(NSTRIP):
                    pending.append((h, s, den, recip, o_raws[s]))
            while pending:
                emit_epilogue(pending.pop(0))
    nc.compile()
    return nc


def get_nc():
    if "nc" not in _nc_cache:
        _nc_cache["nc"] = _build_nc()
    return _nc_cache["nc"]


def _build_tri():
    dk = np.arange(128)[:, None]
    c = np.arange(128)[None, :]
    return np.where(dk <= c, 0.0, NEG).astype(np.float32)


def make_in_maps(qkv):
    import ml_dtypes

    qkv = np.asarray(qkv, dtype=np.float32)
    tri = _build_tri()
    # [128, (a=2) x (m=256)] fp8 weights for the DoubleRow denominator:
    # a single 1.0 column at m == 96 in both pair halves, zeros elsewhere.
    ones8 = np.zeros((128, 512), ml_dtypes.float8_e4m3)
    ones8[:, 96] = 1.0
    ones8[:, 256 + 96] = 1.0
    onesv = np.ones((128, 128), np.float32)
    in_maps = []
    for core in range(NCORES):
        qkvT = np.empty((HPC, 128, 3 * S), np.float16)
        for i in range(HPC):
            bh = core * HPC + i
            b, h = bh // H, bh % H
            qkvT[i, :, 0:S] = qkv[b, :, 0, h, :].T
            qkvT[i, :, S : 2 * S] = qkv[b, :, 1, h, :].T
            qkvT[i, :, 2 * S : 3 * S] = (
                qkv[b, :, 2, h, :]
                .reshape(S // KB, KB, D)
                .transpose(1, 0, 2)
                .reshape(KB, S)
            )
        in_maps.append(
            {"qkvT": qkvT, "tri": tri, "ones8": ones8, "onesv": onesv}
        )
    return in_maps


def assemble_out(results):
    out = np.empty((B, S, H, D), np.float32)
    for core in range(NCORES):
        oTc = results[core]["oT"]  # [HPC, 128, S] bf16
        for i in range(HPC):
            bh = core * HPC + i
            b, h = bh // H, bh % H
            out[b, :, h, :] = oTc[i].astype(np.float32).T
    return out


def kernel(qkv):
    from concourse.bass_utils import run_bass_kernel_spmd

    in_maps = make_in_maps(qkv)
    nc = get_nc()
    res = run_bass_kernel_spmd(nc, in_maps, list(range(NCORES)))
    return assemble_out(res.results)


# revision 23
# speedup vs baseline: 1.0781x; 1.0507x over previous
"""Causal multi-head attention (QKV-packed) on 8 Trainium2 NeuronCores.

Sharding: pure head-parallel. B*H = 32 (batch, head) pairs -> 4 per core,
zero inter-core communication. Flash-style causal attention per head, all
in the "transposed" orientation (k on partitions) so no on-device
transposes are needed:

  - Host pre-lays-out Q^T, K^T as [D=128, S] (fp16, D on partitions) and V
    as k-blocks [128, D] (fp16). Scores for a PAIR of k-blocks land in one
    [128, 1024] PSUM tile (2 banks); one ACT instruction computes
    pt = exp(scale*s - 2) over the written extent. The -2 bias keeps
    exp <= ~45 < 240 (TRN fp8e4 max) and cancels between numerator and
    denominator. Diagonal blocks pack contiguously (t3 at [512,640), t2 at
    [512,768)) so no masked-garbage columns feed the denominator.
  - pt is fp8e4 except strip 0's diagonal pairs (bf16): short softmax rows
    (q < ~100) lack the num/den error cancellation that makes fp8 safe for
    long rows.
  - O^T[d, q] += V_j.T @ pt accumulates in PSUM per 512-col q-strip
    (fp16 x fp8/bf16, 1 col/cycle), then is evacuated to SBUF bf16 at
    strip end to free the bank.
  - Two heads are processed as a PAIR with strips interleaved
    (hA.s0, hB.s3, hA.s1, hB.s2, ...): each adjacent slot-pair has a
    constant amount of matmul work, so the PE never sees a multi-us idle
    stretch (which would re-engage the HAM clock throttle to 1.2 GHz).
  - Both heads' softmax denominators share ONE PSUM bank: strip s of the
    even head accumulates on partition 32s, of the odd head on 16+32s.
    The row is selected by the weight column of an M=128 matmul (fp8
    DoubleRow pairs at 2 cols/cycle for off-diagonal work; normal-rate
    ones-column matmuls for diagonal solo regions and strip 0). Weight
    columns that map to other live rows are exactly 0; never-live rows get
    2^-6 so their denominators stay finite (a 0 would turn the batched
    reciprocal into Inf and poison the K=32 broadcast matmul with 0*Inf).
    One DVE reciprocal per head pair serves all 8 strips.
  - Normalization: K=32 selector matmul broadcasts the reciprocal row into
    the retired den bank, one DVE cast to bf16, one bf16 DVE multiply.
    Output is bf16 (host casts to fp32). Epilogues of a head pair are
    spread one-per-slot across the next pair to avoid DVE pileups.
  - Zero-input warmup matmuls (no DMA dependency) run first so the PE HAM
    clock gate opens (1.2 -> 2.4 GHz) before real data arrives.
"""

import sys

if "/opt/trn_rl_repo" not in sys.path:
    sys.path.insert(0, "/opt/trn_rl_repo")

import numpy as np

B, S, H, D = 2, 2048, 16, 128
NCORES = 8
HPC = (B * H) // NCORES  # heads per core = 4
QS = 512   # q-strip width (PSUM bank)
KB = 128   # k-block (partition dim)
NEG = -1.0e30
SCALE = 1.0 / float(np.sqrt(D))
EXP_BIAS = -2.0
NSTRIP = S // QS  # 4
EPS8 = 0.015625  # 2^-6, min normal e4m3

_nc_cache = {}


def _build_nc():
    import concourse.bass as bass  # noqa: F401
    import concourse.mybir as mybir
    from concourse import bacc
    from concourse.tile import TileContext

    f32 = mybir.dt.float32
    f16 = mybir.dt.float16
    f8 = mybir.dt.float8e4
    bf16 = mybir.dt.bfloat16
    f32r = mybir.dt.float32r
    Exp = mybir.ActivationFunctionType.Exp
    DR = mybir.MatmulPerfMode.DoubleRow

    nc = bacc.Bacc()
    # One packed input per head [128, 3*S] fp16:
    # cols [0,S) = Q^T, [S,2S) = K^T, [2S,3S) = V swizzled so column
    # block j holds the V k-block [128, D] (v[p, j*KB+d] = V[j*KB+p, d]).
    qkvT = nc.declare_dram_parameter("qkvT", [HPC, 128, 3 * S], f16, isOutput=False)
    tri_d = nc.declare_dram_parameter("tri", [128, 128], f32, isOutput=False)
    ones8_d = nc.declare_dram_parameter("ones8", [128, 512], f8, isOutput=False)
    selv_d = nc.declare_dram_parameter("selv", [128, 256], f32r, isOutput=False)
    oT = nc.declare_dram_parameter("oT", [HPC, 128, S], bf16, isOutput=True)

    with TileContext(nc) as tc:
        with (
            nc.allow_low_precision(reason="fp16/fp8/bf16 staging is within tolerance"),
            tc.tile_pool(name="cpool", bufs=1) as cpool,
            tc.tile_pool(name="qkpool", bufs=4) as qkpool,
            tc.tile_pool(name="ptpool", bufs=6) as ptpool,
            tc.tile_pool(name="ptbpool", bufs=4) as ptbpool,
            tc.tile_pool(name="orpool", bufs=16) as orpool,
            tc.tile_pool(name="rcpool", bufs=2) as rcpool,
            tc.tile_pool(name="obpool", bufs=4) as obpool,
            tc.tile_pool(name="scp", bufs=2, space="PSUM") as scp,
            tc.tile_pool(name="pso", bufs=2, space="PSUM") as pso,
            tc.tile_pool(name="psd", bufs=2, space="PSUM") as psd,
        ):
            tri_sb = cpool.tile([128, 128], f32)
            nc.sync.dma_start(out=tri_sb[:], in_=tri_d[:])
            ones8 = cpool.tile([128, 512], f8)
            nc.sync.dma_start(out=ones8[:], in_=ones8_d[:])
            selv = cpool.tile([128, 256], f32r)
            nc.sync.dma_start(out=selv[:], in_=selv_d[:])
            biasc = cpool.tile([128, 1], f32)
            nc.gpsimd.memset(biasc[:], EXP_BIAS)
            # [128, 2, 256] pair view of the den weights (1.0 at m=112)
            o83 = ones8[:, 0:512].rearrange("p (a m) -> p a m", a=2)

            # HAM warmup: zero-input matmuls with no DMA dependency keep the
            # PE busy from the end of the preamble so the clock gate opens
            # (1.2 -> 2.4 GHz) before the first real matmul.
            zsrc = cpool.tile([128, 384], f16)
            nc.scalar.memzero(zsrc[:])
            wps = scp.tile([128, 1024], f32, tag="sc")
            for w in range(14):
                nc.tensor.matmul(
                    wps[:, 0:256],
                    lhsT=zsrc[:, 0:128],
                    rhs=zsrc[:, 128:384],
                    start=True,
                    stop=True,
                )

            def emit_epilogue(ep):
                h, s, row, den_t, recip_t = ep
                o_raw = o_raw_of[(h, s)]
                # broadcast recip row across partitions via a K=32 selector
                # matmul into the retired den bank of this head pair
                wa = 32 * s
                sel = selv[wa : wa + 32, 0:128] if row % 32 == 0 else (
                    selv[wa : wa + 32, 128:256]
                )
                nc.tensor.matmul(
                    den_t[:, :],
                    lhsT=sel,
                    rhs=recip_t[wa : wa + 32, 0:QS],
                    start=True,
                    stop=True,
                    tile_position=(wa, 0),
                )
                rb_sb = obpool.tile([128, QS], bf16, tag="rb_sb")
                nc.vector.tensor_copy(rb_sb[:], den_t[:, :])
                o_sb = obpool.tile([128, QS], bf16, tag="o_sb")
                nc.vector.tensor_mul(o_sb[:], o_raw[:], rb_sb[:])
                nc.sync.dma_start(out=oT[h][:, QS * s : QS * (s + 1)], in_=o_sb[:])

            pending = []
            o_raw_of = {}

            def emit_strip(h, s, qkv_sb, den, row, den_state, pair_last_slot):
                """Emit one q-strip of head h. den_state = [started]."""
                qt = qkv_sb[:, 0:S]
                kt = qkv_sb[:, S : 2 * S]
                vv = qkv_sb[:, 2 * S : 3 * S]
                r = row
                o_ps = pso.tile([128, QS], f32, tag="o_ps")
                q0 = QS * s

                def den_flags(last_of_strip):
                    st = not den_state[0]
                    den_state[0] = True
                    sp = pair_last_slot and last_of_strip
                    return st, sp

                # pairs: (jA, jB, woff, wN, bcol, NB, triA, triB)
                pairs = []
                for p in range(2 * s):
                    pairs.append((2 * p, 2 * p + 1, 0, QS, 512, QS, None, None))
                t0, t1, t2, t3 = 4 * s, 4 * s + 1, 4 * s + 2, 4 * s + 3
                # X = (t0, t3): t0 covers [0,512), t3 at tile [512,640)
                pairs.append((t0, t3, 0, QS, 512, 128, 0, 384))
                # Y = (t1, t2): t1 covers [128,512) at tile [0,384),
                # t2 at tile [512,768); tile [384,512) stays stale and its
                # exp output is never read
                pairs.append((t1, t2, 128, 384, 512, 256, 128, 256))

                npair = len(pairs)
                deferred = None

                def emit_pv_den(p):
                    (pi, jA, jB, woff, wN, bcol, NB, pt, diag, s0d) = p
                    first, last = (pi == 0), (pi == npair - 1)
                    nc.tensor.matmul(
                        o_ps[:, woff : woff + wN],
                        lhsT=vv[:, KB * jA : KB * (jA + 1)],
                        rhs=pt[:, 0:wN],
                        start=first,
                        stop=False,
                    )
                    nc.tensor.matmul(
                        o_ps[:, woff + wN - NB : woff + wN],
                        lhsT=vv[:, KB * jB : KB * (jB + 1)],
                        rhs=pt[:, bcol : bcol + NB],
                        start=False,
                        stop=last,
                    )
                    if not diag:
                        # full pair: DoubleRow over the whole strip
                        st, sp = den_flags(False)
                        nc.tensor.matmul(
                            den[0:128, 0:QS],
                            lhsT=o83[:, :, 112 - r : 240 - r],
                            rhs=pt[:, 0:1024].rearrange("p (a b) -> p a b", a=2),
                            start=st,
                            stop=sp,
                            perf_mode=DR,
                        )
                    elif s0d:
                        # strip 0 diagonal (bf16 pt): two normal-rate
                        # ones-column matmuls per pair
                        st, _ = den_flags(False)
                        nc.tensor.matmul(
                            den[0:128, woff : woff + wN],
                            lhsT=ones8[:, 112 - r : 240 - r],
                            rhs=pt[:, 0:wN],
                            start=st,
                            stop=False,
                        )
                        st, sp = den_flags(last)
                        nc.tensor.matmul(
                            den[0:128, woff + wN - NB : woff + wN],
                            lhsT=ones8[:, 112 - r : 240 - r],
                            rhs=pt[:, bcol : bcol + NB],
                            start=False,
                            stop=sp,
                        )
                    else:
                        # diagonal pair, fp8: solo region (A only) at normal
                        # rate + the overlap region as a DoubleRow pair with
                        # stride (bcol - solo)
                        solo = wN - NB
                        st, _ = den_flags(False)
                        nc.tensor.matmul(
                            den[0:128, woff : woff + solo],
                            lhsT=ones8[:, 112 - r : 240 - r],
                            rhs=pt[:, 0:solo],
                            start=st,
                            stop=False,
                        )
                        st, sp = den_flags(last)
                        pr = pt[:, solo : solo + 2 * (bcol - solo)].rearrange(
                            "p (a b) -> p a b", a=2
                        )
                        nc.tensor.matmul(
                            den[0:128, woff + solo : woff + wN],
                            lhsT=o83[:, :, 112 - r : 240 - r],
                            rhs=pr[:, :, 0:NB],
                            start=False,
                            stop=sp,
                            perf_mode=DR,
                        )

                for pi, (jA, jB, woff, wN, bcol, NB, trA, trB) in enumerate(pairs):
                    diag = pi >= npair - 2
                    s0d = diag and s == 0
                    sc = scp.tile([128, 1024], f32, tag="sc")
                    nc.tensor.matmul(
                        sc[:, 0:wN],
                        lhsT=kt[:, KB * jA : KB * (jA + 1)],
                        rhs=qt[:, q0 + woff : q0 + woff + wN],
                        start=True,
                        stop=True,
                    )
                    bq = q0 + woff + wN - NB
                    nc.tensor.matmul(
                        sc[:, bcol : bcol + NB],
                        lhsT=kt[:, KB * jB : KB * (jB + 1)],
                        rhs=qt[:, bq : bq + NB],
                        start=True,
                        stop=True,
                    )
                    if trA is not None:
                        c = trA - woff
                        nc.vector.tensor_add(
                            sc[:, c : c + 128], sc[:, c : c + 128], tri_sb[:]
                        )
                    if trB is not None:
                        nc.vector.tensor_add(
                            sc[:, bcol : bcol + 128],
                            sc[:, bcol : bcol + 128],
                            tri_sb[:],
                        )
                    ext = bcol + NB
                    if s0d:
                        pt = ptbpool.tile([128, 1024], bf16, tag="ptb")
                    else:
                        pt = ptpool.tile([128, 1024], f8, tag="pt")
                    nc.scalar.activation(
                        pt[:, 0:ext], sc[:, 0:ext], Exp, bias=biasc[:], scale=SCALE
                    )
                    # defer PV/den one pair so the PE always has the next
                    # pair's score matmuls queued ahead of work that waits
                    # on the ACT output
                    if deferred is not None:
                        emit_pv_den(deferred)
                    deferred = (pi, jA, jB, woff, wN, bcol, NB, pt, diag, s0d)
                emit_pv_den(deferred)
                # evacuate O^T early (frees the PSUM bank; bf16 is fine for
                # the un-normalized accumulator)
                o_raw = orpool.tile([128, QS], bf16, tag="o_raw")
                nc.vector.tensor_copy(o_raw[:], o_ps[:])
                o_raw_of[(h, s)] = o_raw

            for pr_i in range(HPC // 2):
                hA, hB = 2 * pr_i, 2 * pr_i + 1
                sbufs = {}
                for h in (hA, hB):
                    qkv_sb = qkpool.tile([128, 3 * S], f16, tag="qkv_sb")
                    sbufs[h] = qkv_sb
                    if h == 0:
                        # split the first head's load so the first matmuls
                        # can start early
                        for c0, c1 in (
                            (S, S + 512),          # K^T blocks 0-3
                            (0, 512),              # Q^T strip 0
                            (2 * S, 2 * S + 512),  # V blocks 0-3
                            (512, S),              # Q^T rest
                            (S + 512, 2 * S),      # K^T rest
                            (2 * S + 512, 3 * S),  # V rest
                        ):
                            nc.sync.dma_start(
                                out=qkv_sb[:, c0:c1], in_=qkvT[h][:, c0:c1]
                            )
                    elif h == 1:
                        # second head of the first pair starts at strip 3:
                        # K^T fully, then Q^T strip 3, then V, then the rest
                        for c0, c1 in (
                            (S, 2 * S),            # K^T
                            (3 * QS, S),           # Q^T strip 3
                            (2 * S, 3 * S),        # V
                            (0, 3 * QS),           # Q^T rest
                        ):
                            nc.sync.dma_start(
                                out=qkv_sb[:, c0:c1], in_=qkvT[h][:, c0:c1]
                            )
                    else:
                        nc.sync.dma_start(out=qkv_sb[:], in_=qkvT[h])

                den = psd.tile([128, QS], f32, tag="den")
                den_state = [False]
                slots = []
                for s in range(NSTRIP):
                    slots.append((hA, s))
                    slots.append((hB, NSTRIP - 1 - s))
                for si, (h, s) in enumerate(slots):
                    row = 32 * s + (16 if h % 2 else 0)
                    emit_strip(
                        h, s, sbufs[h], den, row, den_state, si == len(slots) - 1
                    )
                    if pending:
                        emit_epilogue(pending.pop(0))

                # one batched reciprocal per head pair (rows 0,16,...,112
                # hold the 8 strips' denominators)
                recip = rcpool.tile([128, QS], f32r, tag="recip")
                nc.vector.reciprocal(recip[:], den[:])
                for h in (hA, hB):
                    for s in range(NSTRIP):
                        row = 32 * s + (16 if h % 2 else 0)
                        pending.append((h, s, row, den, recip))
            while pending:
                emit_epilogue(pending.pop(0))
    nc.compile()
    return nc


def get_nc():
    if "nc" not in _nc_cache:
        _nc_cache["nc"] = _build_nc()
    return _nc_cache["nc"]


def _build_tri():
    dk = np.arange(128)[:, None]
    c = np.arange(128)[None, :]
    return np.where(dk <= c, 0.0, NEG).astype(np.float32)


def make_in_maps(qkv):
    import ml_dtypes

    qkv = np.asarray(qkv, dtype=np.float32)
    tri = _build_tri()
    # Denominator weights, [128, (a=2) x (m=256)] fp8: column m selects the
    # output partition of an M=128 matmul sliced at [112-r, 240-r). 1.0 at
    # m=112 (the target row r); exactly 0 at other m = 0 mod 16 (those map
    # onto other LIVE den rows); 2^-6 elsewhere so never-live rows hold a
    # finite denominator (reciprocal of 0 would be Inf, and 0*Inf = NaN in
    # the K=32 broadcast matmul).
    m = np.arange(256)
    col = np.where(m % 16 == 0, 0.0, EPS8).astype(np.float32)
    col[112] = 1.0
    ones8 = np.broadcast_to(
        np.concatenate([col, col])[None, :], (128, 512)
    ).astype(ml_dtypes.float8_e4m3)
    # K=32 selector weights for the reciprocal broadcast: partition p of
    # cols [0,128) is 1.0 iff p % 32 == 0; of cols [128,256) iff p % 32 == 16.
    p = np.arange(128)[:, None]
    selv = np.concatenate(
        [
            np.where(p % 32 == 0, 1.0, 0.0).repeat(128, axis=1),
            np.where(p % 32 == 16, 1.0, 0.0).repeat(128, axis=1),
        ],
        axis=1,
    ).astype(np.float32)
    in_maps = []
    for core in range(NCORES):
        qkvT = np.empty((HPC, 128, 3 * S), np.float16)
        for i in range(HPC):
            bh = core * HPC + i
            b, h = bh // H, bh % H
            qkvT[i, :, 0:S] = qkv[b, :, 0, h, :].T
            qkvT[i, :, S : 2 * S] = qkv[b, :, 1, h, :].T
            qkvT[i, :, 2 * S : 3 * S] = (
                qkv[b, :, 2, h, :]
                .reshape(S // KB, KB, D)
                .transpose(1, 0, 2)
                .reshape(KB, S)
            )
        in_maps.append(
            {"qkvT": qkvT, "tri": tri, "ones8": ones8, "selv": selv}
        )
    return in_maps


def assemble_out(results):
    out = np.empty((B, S, H, D), np.float32)
    for core in range(NCORES):
        oTc = results[core]["oT"]  # [HPC, 128, S] bf16
        for i in range(HPC):
            bh = core * HPC + i
            b, h = bh // H, bh % H
            out[b, :, h, :] = oTc[i].astype(np.float32).T
    return out


def kernel(qkv):
    from concourse.bass_utils import run_bass_kernel_spmd

    in_maps = make_in_maps(qkv)
    nc = get_nc()
    res = run_bass_kernel_spmd(nc, in_maps, list(range(NCORES)))
    return assemble_out(res.results)
